# revision 18
# baseline (speedup 1.0000x reference)
"""Multi-head attention (b=4, n=2048, dim=512, h=8, dh=64) on 8 trn2 cores.

Sharding: core c handles batch b=c//2 and query rows
[half*1024, (half+1)*1024) with half=c%2. K/V (from x_prev) are computed
redundantly on both of a batch's cores (cheap vs. attention). No collectives.

Per-core kernel (bf16 operands, fp32 PSUM accumulation):
  QT[inner, nq]  = w_q-tiles  (lhsT) @ x^T          (q in transposed layout)
  KT[inner, nk]  = w_kv-tiles (lhsT) @ x_prev^T
  V [nk, inner]  = x_prev^T-tiles (lhsT) @ w_kv[:, v]  (natural layout,
                                                        + ones column/head)
  ST[j, i]       = KT_h-tile (lhsT, K=dh=64) @ QT_h  (scores transposed;
                   two heads row-tiled in the PE at partitions 0/64)
  PT             = exp(ST * scale)    (no max subtraction: |s*scale| < ~8)
  PVT[q, dh+1]   = sum_j PT_h-tile (lhsT, K=128 keys) @ V_h|ones
                   (transposed-PV form: 65-wide moving operand instead of
                    512-wide -> half the PE row count; col dh = l = sum_j P)
  r[q]           = 1/l  -> AO[q, h dh] = PVT[:, 0:dh] * r   (per-partition
                   scalar broadcast on DVE; no partition-broadcast needed)
  AOT            = PE-transpose(AO) per 128x128 tile  -> [inner, nq] layout
  out[i, d]      = sum_k AOT-tiles (lhsT, K=128) @ w_out-tiles + ones @ b_out

Emission is software-pipelined: per (pair, key-tile) slot the scheduler
emits ST+exp, then drains lagged PVT groups and background work (KT/QT/V
projections, AO transposes, output projection) into the PE slack so the
Activation engine (exp ~= 133us busy, the critical path) is never starved.
Background work is chunked (one psum tile per chunk); chunks may be
reordered by deadline but never interleave with each other, so the tag-
rotated psum buffers see a clean write->drain->write order.
"""

import numpy as np
import ml_dtypes

B, N, DIM = 4, 2048, 512
H, DH, INNER = 8, 64, 512
NCORES = 8

_BUILT = None


def build_module(compile_module=True, reps=1, stub=frozenset()):
    """Build the per-core attention module (nq=1024, nk=2048 hardcoded).
    reps>1 repeats the whole compute body (timing calibration only)."""
    import concourse.mybir as mybir
    import concourse.tile as tile
    from concourse import bacc

    CDT = mybir.dt.bfloat16
    FDT = mybir.dt.float32
    Exp = mybir.ActivationFunctionType.Exp

    dim, h = DIM, H
    nq, nk = N // 2, N
    inner = h * DH
    nkt = dim // 128          # contraction tiles for projections
    npr = h // 2              # head pairs (= inner // 128 slices of KT/QT)
    nj = nk // 128            # key tiles
    nqc = nq // 512           # query chunks
    VW = DH + 1               # 65: per-head v columns + ones column
    scale = DH ** -0.5

    nc = bacc.Bacc("TRN2", target_bir_lowering=False, debug=False,
                   num_devices=NCORES)

    xt_d = nc.declare_dram_parameter("xt", [dim, nq], CDT, isOutput=False)
    xpt_d = nc.declare_dram_parameter("xpt", [dim, nk], CDT, isOutput=False)
    wq_d = nc.declare_dram_parameter("wq", [dim, inner], CDT, isOutput=False)
    wkv_d = nc.declare_dram_parameter("wkv", [dim, 2 * inner], CDT,
                                      isOutput=False)
    wout_d = nc.declare_dram_parameter("wout", [inner, dim], CDT,
                                       isOutput=False)
    bout_d = nc.declare_dram_parameter("bout", [1, dim], CDT, isOutput=False)
    id_d = nc.declare_dram_parameter("ident", [128, 128], CDT, isOutput=False)
    out_d = nc.declare_dram_parameter("out", [nq, dim], FDT, isOutput=True)

    import contextlib
    with tile.TileContext(nc) as tc, contextlib.ExitStack() as stack:
        consts = stack.enter_context(tc.tile_pool(name="consts", bufs=1))
        acts = stack.enter_context(tc.tile_pool(name="acts", bufs=1))
        st_scope = stack.enter_context(
            tc.tile_pool(name="st_ps", bufs=2, space="PSUM"))
        acc_scope = stack.enter_context(
            tc.tile_pool(name="acc_ps", bufs=1, space="PSUM"))
        mm_scope = stack.enter_context(
            tc.tile_pool(name="mm_ps", bufs=2, space="PSUM"))
        pt_pool = stack.enter_context(tc.tile_pool(name="pt", bufs=2))
        lr_pool = stack.enter_context(tc.tile_pool(name="lr", bufs=4))

        # ---- constants / weights ----------------------------------------
        wq_sb = consts.tile([128, nkt, inner], CDT)
        wkv_sb = consts.tile([128, nkt, 2 * inner], CDT)
        wout_sb = consts.tile([128, nkt, dim], CDT)
        bout_sb = consts.tile([1, dim], CDT)
        id_sb = consts.tile([128, 128], CDT)
        ones_sb = consts.tile([1, 128], CDT)

        xt_sb = acts.tile([128, nkt, nq], CDT)
        xpt_sb = acts.tile([128, nkt, nk], CDT)
        wq_r = wq_d.ap().rearrange("(t p) o -> p t o", p=128)
        wkv_r = wkv_d.ap().rearrange("(t p) o -> p t o", p=128)
        wout_r = wout_d.ap().rearrange("(t p) o -> p t o", p=128)
        xt_r = xt_d.ap().rearrange("(t p) n -> p t n", p=128)
        xpt_r = xpt_d.ap().rearrange("(t p) n -> p t n", p=128)
        # DMA loads spread over queues, dependency-first. Pool has the
        # cheapest DMA issue cost, so it takes the many small xpt chunks;
        # kt0c0 needs xpt cols 0:512 + wkv K-half, qt0c0 needs xt cols
        # 0:512 + wq -- those go first on their queues.
        # Consolidated multi-dim DMAs (few issues), dependency-first.
        # SP: kt0c0 critical path, then the rest of xpt.
        nc.sync.dma_start(out=xpt_sb[:, :, 0:512], in_=xpt_r[:, :, 0:512])
        nc.sync.dma_start(out=wkv_sb[:, :, 0:inner],
                          in_=wkv_r[:, :, 0:inner])
        nc.sync.dma_start(out=wkv_sb[:, :, inner:2 * inner],
                          in_=wkv_r[:, :, inner:2 * inner])
        nc.sync.dma_start(out=xpt_sb[:, :, 512:1024],
                          in_=xpt_r[:, :, 512:1024])
        nc.sync.dma_start(out=xpt_sb[:, :, 1024:2048],
                          in_=xpt_r[:, :, 1024:2048])
        nc.sync.dma_start(out=id_sb[:, :], in_=id_d.ap())
        nc.sync.dma_start(out=bout_sb[:, :], in_=bout_d.ap())
        # Act: qt0c0 critical path, then free for exp.
        nc.scalar.dma_start(out=wq_sb[:, :, :], in_=wq_r[:, :, :])
        nc.scalar.dma_start(out=xt_sb[:, :, 0:512], in_=xt_r[:, :, 0:512])
        nc.vector.memset(ones_sb[:, :], 1.0)

        qt_sb = acts.tile([128, npr, nq], CDT)    # [inner-slice, nq]
        kt_sb = acts.tile([128, npr, nk], CDT)    # [inner-slice, nk]
        v_sb = acts.tile([128, nj, h * VW], CDT)  # [key-tile, h*(dh+1)]
        ao_sb = acts.tile([128, nq // 128, inner], CDT)   # [q, qtile, inner]
        aotT_sb = acts.tile([128, nkt, nq], CDT)  # [inner-slice, nq]

        for hh in range(h):  # ones columns of V
            nc.vector.memset(v_sb[:, :, hh * VW + DH:hh * VW + DH + 1], 1.0)
        if stub:
            nc.vector.memset(ao_sb[:, :, :], 0.5)
            nc.vector.memset(pt_dummy_guard[:, :], 1.0) if False else None

        # Warm the Exp activation table while DMAs stream in.
        warm_sb = consts.tile([1, 1], FDT)
        nc.scalar.activation(out=warm_sb[:, :], in_=ones_sb[0:1, 0:1],
                             func=Exp, scale=scale)

        # ================= emission scheduler =============================
        PAIRS = [(c, p) for c in range(nqc) for p in range(npr)]
        EST = {"st": 438.0, "pv": 250.0, "mm": 220.0, "tr": 70.0,
               "exp": 1060.0}
        MARGIN = 150.0
        carry = []   # unfinished background chunks handed to the next rep

        for _rep in range(reps):
            state = {"pe": 0.0, "act": 0.0, "v_emitted": 0}
            exp_est = {}

            # ---- background chunks (lazy psum tile, thunk list) ---------
            def kt_thunks(s, cc):
                cell = {}
                for k in range(nkt):
                    def mm(k=k, s=s, cc=cc, cell=cell):
                        if k == 0:
                            cell["ps"] = mm_scope.tile([128, 512], FDT,
                                                       tag="mm", name="mmps")
                        ps = cell["ps"]
                        nc.tensor.matmul(
                            ps[:, :],
                            lhsT=wkv_sb[:, k, s * 128:(s + 1) * 128],
                            rhs=xpt_sb[:, k, cc * 512:(cc + 1) * 512],
                            start=(k == 0), stop=(k == nkt - 1))
                        if k == nkt - 1:
                            nc.vector.tensor_copy(
                                out=kt_sb[:, s, cc * 512:(cc + 1) * 512],
                                in_=ps[:, :])
                    yield (mm, EST["mm"])

            def qt_thunks(s, cc):
                cell = {}
                for k in range(nkt):
                    def mm(k=k, s=s, cc=cc, cell=cell):
                        if k == 0:
                            cell["ps"] = mm_scope.tile([128, 512], FDT,
                                                       tag="mm", name="mmps")
                        ps = cell["ps"]
                        nc.tensor.matmul(
                            ps[:, :],
                            lhsT=wq_sb[:, k, s * 128:(s + 1) * 128],
                            rhs=xt_sb[:, k, cc * 512:(cc + 1) * 512],
                            start=(k == 0), stop=(k == nkt - 1))
                        if k == nkt - 1:
                            nc.vector.tensor_copy(
                                out=qt_sb[:, s, cc * 512:(cc + 1) * 512],
                                in_=ps[:, :])
                    yield (mm, EST["mm"])

            def v_thunks(j):
                cell = {}
                for k in range(nkt):
                    def mm(k=k, j=j, cell=cell):
                        if k == 0:
                            cell["ps"] = mm_scope.tile([128, 512], FDT,
                                                       tag="mm", name="mmps")
                        ps = cell["ps"]
                        nc.tensor.matmul(
                            ps[:, :],
                            lhsT=xpt_sb[:, k, j * 128:(j + 1) * 128],
                            rhs=wkv_sb[:, k, inner:2 * inner],
                            start=(k == 0), stop=(k == nkt - 1))
                        if k == nkt - 1:
                            nc.vector.tensor_copy(
                                out=v_sb[:, j, :].rearrange(
                                    "p (g x) -> p g x", x=VW)[:, :, 0:DH],
                                in_=ps[:, :].rearrange(
                                    "p (g x) -> p g x", x=DH))
                            state["v_emitted"] = j + 1
                    yield (mm, EST["mm"])

            def tr_thunks(c, p):
                if "noout" in stub:
                    return
                cell = {}
                for qq in range(4):
                    def mm(qq=qq, c=c, p=p, cell=cell):
                        if qq == 0:
                            cell["ps"] = mm_scope.tile([128, 512], CDT,
                                                       tag="mm", name="trps")
                        ps = cell["ps"]
                        nc.tensor.matmul(
                            ps[:, qq * 128:(qq + 1) * 128],
                            lhsT=ao_sb[:, c * 4 + qq, p * 128:(p + 1) * 128],
                            rhs=id_sb[:, :], is_transpose=True,
                            start=(qq == 0), stop=(qq == 3))
                        if qq == 3:
                            nc.vector.tensor_copy(
                                out=aotT_sb[:, p, c * 512:(c + 1) * 512],
                                in_=ps[:, :])
                    yield (mm, EST["tr"])

            def out_thunks(c, t):
                if "noout" in stub:
                    def sentinel(t=t):
                        fo = lr_pool.tile([128, dim], FDT, tag="fo", bufs=2,
                                          name="fo")
                        nc.vector.tensor_copy(out=fo[:, :],
                                              in_=st_scope.tile(
                                                  [128, 1024], FDT,
                                                  tag="st",
                                                  name="stx")[:, 0:512])
                        nc.sync.dma_start(
                            out=out_d.ap()[t * 128:(t + 1) * 128, :],
                            in_=fo[:, :])
                    yield (sentinel, EST["mm"])
                    return
                cell = {}
                for k in range(nkt):
                    def mm(k=k, t=t, cell=cell):
                        if k == 0:
                            cell["ps"] = mm_scope.tile([128, 512], FDT,
                                                       tag="mm", name="mmps")
                        ps = cell["ps"]
                        nc.tensor.matmul(
                            ps[:, :],
                            lhsT=aotT_sb[:, k, t * 128:(t + 1) * 128],
                            rhs=wout_sb[:, k, :],
                            start=(k == 0), stop=False)
                    yield (mm, EST["mm"])

                def bias(t=t, cell=cell):
                    ps = cell["ps"]
                    nc.tensor.matmul(ps[:, :], lhsT=ones_sb[:, :],
                                     rhs=bout_sb[:, :], start=False,
                                     stop=True)
                    fo = lr_pool.tile([128, dim], FDT, tag="fo", bufs=2,
                                      name="fo")
                    nc.vector.tensor_copy(out=fo[:, :], in_=ps[:, :])
                    deng = nc.sync if t % 2 == 0 else nc.scalar
                    deng.dma_start(
                        out=out_d.ap()[t * 128:(t + 1) * 128, :], in_=fo[:, :])
                yield (bias, EST["mm"])

            # background FIFO of chunks: [deadline_slot, [thunks], earliest]
            bg = []
            cur = {"chunk": None}  # chunk being emitted (never interleaved)

            def push(deadline, gen, earliest=0):
                bg.append([deadline, list(gen), earliest])

            first = _rep == 0
            for dl, thunks, early in carry:   # previous rep's leftovers
                bg.append([dl, thunks, 0])
            carry.clear()
            # In rep 0 the input DMAs land over the first ~10us, so hold
            # chunks back until their operands exist (earliest slot).
            push(4, kt_thunks(0, 1), 2 if first else 0)
            push(14, v_thunks(0), 1 if first else 0)
            push(14, v_thunks(1), 1 if first else 0)
            push(8, kt_thunks(0, 2), 3 if first else 0)
            push(14, v_thunks(2), 1 if first else 0)
            push(14, v_thunks(3), 1 if first else 0)
            push(12, kt_thunks(0, 3), 3 if first else 0)
            push(18, v_thunks(4), 2 if first else 0)
            push(18, v_thunks(5), 2 if first else 0)
            push(18, v_thunks(6), 2 if first else 0)
            push(16, qt_thunks(1, 0))
            push(16, kt_thunks(1, 0))
            push(22, v_thunks(7), 2 if first else 0)
            push(24, v_thunks(8), 5 if first else 0)
            push(20, kt_thunks(1, 1))
            push(26, v_thunks(9), 5 if first else 0)
            push(28, v_thunks(10), 5 if first else 0)
            push(24, kt_thunks(1, 2))
            push(29, v_thunks(11), 5 if first else 0)
            push(30, v_thunks(12), 6 if first else 0)
            push(28, kt_thunks(1, 3))
            push(30, v_thunks(13), 6 if first else 0)
            push(31, v_thunks(14), 6 if first else 0)
            push(31, v_thunks(15), 6 if first else 0)
            push(32, qt_thunks(2, 0))
            push(32, kt_thunks(2, 0))
            push(36, kt_thunks(2, 1))
            push(40, kt_thunks(2, 2))
            push(44, kt_thunks(2, 3))
            push(48, qt_thunks(3, 0))
            push(48, kt_thunks(3, 0))
            push(52, kt_thunks(3, 1))
            push(56, kt_thunks(3, 2))
            push(60, kt_thunks(3, 3))
            push(64, qt_thunks(0, 1))
            push(80, qt_thunks(1, 1))
            push(96, qt_thunks(2, 1))
            push(108, qt_thunks(3, 1))

            def pop_bg_thunk(slot):
                """Emit one background thunk; returns False if none exists.
                Chunks are atomic: once started, later pops continue it."""
                if cur["chunk"] is None:
                    elig = [i for i, e in enumerate(bg) if e[2] <= slot]
                    if not elig:
                        return False
                    due_i = next((i for i in elig if bg[i][0] <= slot + 1),
                                 None)
                    cur["chunk"] = bg.pop(due_i if due_i is not None
                                          else elig[0])
                fn, est = cur["chunk"][1].pop(0)
                fn()
                state["pe"] += est
                if not cur["chunk"][1]:
                    cur["chunk"] = None
                return True

            def bg_due(slot):
                if cur["chunk"] is not None and cur["chunk"][0] <= slot + 1:
                    return True
                return any(e[0] <= slot + 1 and e[2] <= slot for e in bg)

            # ---- PV stream ----------------------------------------------
            pair_pt = {}
            pair_acc = {}
            pv_queue = [(k, j) for k in range(len(PAIRS)) for j in range(nj)]
            pv_idx = [0]

            def emit_pv(k, j):
                if "nopv" in stub:
                    state["pe"] += EST["pv"]
                    if j == 0:
                        pair_acc[k] = None
                    return
                c, p = PAIRS[k]
                if j == 0:
                    pair_acc[k] = (
                        acc_scope.tile([128, 4, 128], FDT, tag="acc0",
                                       name="acc0"),
                        acc_scope.tile([128, 4, 128], FDT, tag="acc1",
                                       name="acc1"))
                pt = pair_pt[k]
                acc0, acc1 = pair_acc[k]
                # One psum accumulation group per acc bank: start marks the
                # whole 2KB zero region; later qq sub-ranges first-write via
                # the pending-zero overwrite, so only (j0,qq0) starts and
                # (j15,qq3) stops.
                for hh, acc in ((0, acc0), (1, acc1)):
                    h_abs = 2 * p + hh
                    for qq in range(4):
                        nc.tensor.matmul(
                            acc[:, qq, 0:VW],
                            lhsT=pt[:, j, hh * 512 + qq * 128:
                                    hh * 512 + (qq + 1) * 128],
                            rhs=v_sb[:, j, h_abs * VW:(h_abs + 1) * VW],
                            start=(j == 0 and qq == 0),
                            stop=(j == nj - 1 and qq == 3))
                state["pe"] += EST["pv"]

            def emit_normalize(k, slot):
                c, p = PAIRS[k]
                if "nopv" in stub:
                    del pair_acc[k]
                    del pair_pt[k]
                    push(slot + 8, tr_thunks(c, p))
                    if p == npr - 1:
                        for t in range(4):
                            push(slot + 10 + 3 * t, out_thunks(c, 4 * c + t))
                    return
                acc0, acc1 = pair_acc[k]
                for hh, acc in ((0, acc0), (1, acc1)):
                    eng = nc.vector
                    for qq in range(4):
                        r = lr_pool.tile([128, 1], FDT, tag="r", name="rrec")
                        nc.vector.reciprocal(out=r[:, :],
                                             in_=acc[:, qq, DH:DH + 1])
                        eng.tensor_scalar_mul(
                            ao_sb[:, c * 4 + qq,
                                  (2 * p + hh) * DH:(2 * p + hh + 1) * DH],
                            acc[:, qq, 0:DH], r[:, :])
                del pair_acc[k]
                del pair_pt[k]
                push(slot + 8, tr_thunks(c, p))
                if p == npr - 1:
                    for t in range(4):
                        push(slot + 10 + 3 * t, out_thunks(c, 4 * c + t))

            def drain_pv(slot, force_pair_upto=None):
                while pv_idx[0] < len(pv_queue):
                    k, j = pv_queue[pv_idx[0]]
                    if (k, j) not in exp_est:
                        break
                    if state["v_emitted"] <= j:
                        break
                    forced = (force_pair_upto is not None
                              and k <= force_pair_upto)
                    if not forced and (
                            exp_est[(k, j)] > state["pe"] - 50.0
                            or state["pe"] >= state["act"] - MARGIN):
                        break
                    emit_pv(k, j)
                    pv_idx[0] += 1
                    if j == nj - 1:
                        emit_normalize(k, slot)

            # ---- prologue: kt0 chunk0 + qt0 chunk0 ----------------------
            # (for reps>0 these were pushed into the previous rep's tail bg)
            if first:
                for fn, est in list(kt_thunks(0, 0)) + list(qt_thunks(0, 0)):
                    fn()
                    state["pe"] += est

            # ---- main slot loop -----------------------------------------
            for k, (c, p) in enumerate(PAIRS):
                if k == len(PAIRS) - 1 and _rep < reps - 1:
                    # next rep's first kt/qt chunks ride this rep's last
                    # pair so the rep boundary has no dead PE window
                    push(16 * k + 6, kt_thunks(0, 0))
                    push(16 * k + 8, qt_thunks(0, 0))
                if k == 1 and first:
                    # deferred non-critical loads; SP queue is free by now
                    nc.sync.dma_start(out=xt_sb[:, :, 512:1024],
                                      in_=xt_r[:, :, 512:1024])
                    nc.sync.dma_start(out=wout_sb[:, :, :],
                                      in_=wout_r[:, :, :])
                if k >= 2:  # pt buffer rotation (bufs=2)
                    drain_pv(16 * k, force_pair_upto=k - 2)
                    head = (pv_queue[pv_idx[0]][0]
                            if pv_idx[0] < len(pv_queue) else len(PAIRS))
                    assert head > k - 2, "PV stream fell behind pt rotation"
                pair_pt[k] = pt_pool.tile([128, nj, 1024], CDT, tag="pt",
                                          name="ptbuf")
                pt = pair_pt[k]
                for j in range(nj):
                    slot = 16 * k + j
                    st = st_scope.tile([128, 1024], FDT, tag="st")
                    nc.tensor.matmul(
                        st[:, 0:512],
                        lhsT=kt_sb[0:64, p, j * 128:(j + 1) * 128],
                        rhs=qt_sb[0:64, p, c * 512:(c + 1) * 512],
                        start=True, stop=True)
                    nc.tensor.matmul(
                        st[:, 512:1024],
                        lhsT=kt_sb[64:128, p, j * 128:(j + 1) * 128],
                        rhs=qt_sb[64:128, p, c * 512:(c + 1) * 512],
                        start=True, stop=True)
                    state["pe"] += EST["st"]
                    if "noexp" not in stub:
                        nc.scalar.activation(out=pt[:, j, :], in_=st[:, :],
                                             func=Exp, scale=scale)
                    else:
                        nc.vector.tensor_copy(out=pt[:, j, 0:64],
                                              in_=st[:, 0:64])
                    state["act"] = max(state["act"],
                                       state["pe"] + 100.0) + EST["exp"]
                    exp_est[(k, j)] = state["act"]
                    # fill PE slack: overdue bg first, then PV, then bg
                    while bg_due(slot):
                        if not pop_bg_thunk(slot):
                            break
                    drain_pv(slot)
                    while (state["pe"] < state["act"] - MARGIN
                           and pop_bg_thunk(slot)):
                        drain_pv(slot)

            # ---- tail flush ---------------------------------------------
            final_slot = 16 * len(PAIRS)
            drain_pv(final_slot, force_pair_upto=len(PAIRS) - 1)
            assert pv_idx[0] == len(pv_queue), "unemitted PV groups"
            if _rep < reps - 1:
                # hand the remaining chunks (last tr + out c1) to the next
                # rep so they overlap its first exps instead of a dead tail
                if cur["chunk"] is not None:
                    while cur["chunk"] is not None:  # finish current chunk
                        fn, est = cur["chunk"][1].pop(0)
                        fn()
                        if not cur["chunk"][1]:
                            cur["chunk"] = None
                for i, e in enumerate(bg):
                    carry.append([2 + 2 * i, e[1], 0])
                bg.clear()
            else:
                while pop_bg_thunk(10 ** 9):
                    pass
                drain_pv(final_slot, force_pair_upto=len(PAIRS) - 1)
                while pop_bg_thunk(10 ** 9):
                    pass
                assert not bg and cur["chunk"] is None, "unemitted background"

    if compile_module:
        nc.compile()
    return nc


def host_inputs(x, x_prev, w_q, w_kv, w_out, b_out, ncores=NCORES):
    """Shard + lay out the full inputs into per-core input maps."""
    bf16 = ml_dtypes.bfloat16
    b, n, dim = x.shape
    nq = (b * n) // ncores
    halves = ncores // b
    wq = np.ascontiguousarray(w_q).astype(bf16)
    wkv = np.ascontiguousarray(w_kv).astype(bf16)
    wout = np.ascontiguousarray(w_out).astype(bf16)
    bout = np.ascontiguousarray(b_out).reshape(1, dim).astype(bf16)
    ident = np.eye(128, dtype=bf16)
    in_maps = []
    for c in range(ncores):
        bb, half = c // halves, c % halves
        xt = np.ascontiguousarray(
            x[bb, half * nq:(half + 1) * nq, :].T).astype(bf16)
        xpt = np.ascontiguousarray(x_prev[bb].T).astype(bf16)
        in_maps.append(dict(xt=xt, xpt=xpt, wq=wq, wkv=wkv, wout=wout,
                            bout=bout, ident=ident))
    return in_maps


def _get_module():
    global _BUILT
    if _BUILT is None:
        _BUILT = build_module()
    return _BUILT


def kernel(x, x_prev, w_q, w_kv, w_out, b_out):
    from concourse.bass_utils import run_bass_kernel_spmd

    nc = _get_module()
    in_maps = host_inputs(x, x_prev, w_q, w_kv, w_out, b_out)
    res = run_bass_kernel_spmd(nc, in_maps, core_ids=list(range(NCORES)))

    nq = N // 2
    out = np.empty((B, N, DIM), np.float32)
    for c in range(NCORES):
        b, half = c // 2, c % 2
        out[b, half * nq:(half + 1) * nq, :] = res.results[c]["out"]
    return out


# revision 19
# speedup vs baseline: 2.3098x; 2.3098x over previous
"""Multi-head attention (b=4, n=2048, dim=512, h=8, dh=64) on 8 trn2 cores.

Sharding: core c handles batch b=c//2 and query rows
[half*1024, (half+1)*1024) with half=c%2. K/V (from x_prev) are computed
redundantly on both of a batch's cores (cheap vs. attention). No collectives.

Per-core kernel (bf16 operands, fp32 PSUM accumulation):
  QT[inner, nq]  = w_q-tiles  (lhsT) @ x^T          (q in transposed layout)
  KT[inner, nk]  = w_kv-tiles (lhsT) @ x_prev^T
  V [nk, inner]  = x_prev^T-tiles (lhsT) @ w_kv[:, v]  (natural layout,
                                                        + ones column/head)
  ST[j, i]       = KT_h-tile (lhsT, K=dh=64) @ QT_h  (scores transposed;
                   two heads row-tiled in the PE at partitions 0/64)
  PT             = exp(ST * scale)    (no max subtraction: |s*scale| < ~8)
  PVT[q, dh+1]   = sum_j PT_h-tile (lhsT, K=128 keys) @ V_h|ones
                   (transposed-PV form: 65-wide moving operand instead of
                    512-wide -> half the PE row count; col dh = l = sum_j P)
  r[q]           = 1/l  -> AO[q, h dh] = PVT[:, 0:dh] * r   (per-partition
                   scalar broadcast on DVE; no partition-broadcast needed)
  AOT            = PE-transpose(AO) per 128x128 tile  -> [inner, nq] layout
  out[i, d]      = sum_k AOT-tiles (lhsT, K=128) @ w_out-tiles + ones @ b_out

Emission is software-pipelined: per (pair, key-tile) slot the scheduler
emits ST+exp, then drains lagged PVT groups and background work (KT/QT/V
projections, AO transposes, output projection) into the PE slack so the
Activation engine (exp ~= 133us busy, the critical path) is never starved.
Background work is chunked (one psum tile per chunk); chunks may be
reordered by deadline but never interleave with each other, so the tag-
rotated psum buffers see a clean write->drain->write order.
"""

import numpy as np
import ml_dtypes

B, N, DIM = 4, 2048, 512
H, DH, INNER = 8, 64, 512
NCORES = 8

_BUILT = None


def build_module(compile_module=True, reps=1, stub=frozenset()):
    """Build the per-core attention module (nq=1024, nk=2048 hardcoded).
    reps>1 repeats the whole compute body (timing calibration only)."""
    import concourse.mybir as mybir
    import concourse.tile as tile
    from concourse import bacc

    CDT = mybir.dt.bfloat16
    FDT = mybir.dt.float32
    Exp = mybir.ActivationFunctionType.Exp

    dim, h = DIM, H
    nq, nk = N // 2, N
    inner = h * DH
    nkt = dim // 128          # contraction tiles for projections
    npr = h // 2              # head pairs (= inner // 128 slices of KT/QT)
    nj = nk // 128            # key tiles
    nqc = nq // 512           # query chunks
    VW = DH + 1               # 65: per-head v columns + ones column
    scale = DH ** -0.5

    nc = bacc.Bacc("TRN2", target_bir_lowering=False, debug=False,
                   num_devices=NCORES)

    xt_d = nc.declare_dram_parameter("xt", [dim, nq], CDT, isOutput=False)
    xpt_d = nc.declare_dram_parameter("xpt", [dim, nk], CDT, isOutput=False)
    wq_d = nc.declare_dram_parameter("wq", [dim, inner], CDT, isOutput=False)
    wkv_d = nc.declare_dram_parameter("wkv", [dim, 2 * inner], CDT,
                                      isOutput=False)
    wout_d = nc.declare_dram_parameter("wout", [inner, dim], CDT,
                                       isOutput=False)
    bout_d = nc.declare_dram_parameter("bout", [1, dim], CDT, isOutput=False)
    id_d = nc.declare_dram_parameter("ident", [128, 128], CDT, isOutput=False)
    out_d = nc.declare_dram_parameter("out", [nq, dim], FDT, isOutput=True)

    import contextlib
    with tile.TileContext(nc) as tc, contextlib.ExitStack() as stack:
        consts = stack.enter_context(tc.tile_pool(name="consts", bufs=1))
        acts = stack.enter_context(tc.tile_pool(name="acts", bufs=1))
        st_scope = stack.enter_context(
            tc.tile_pool(name="st_ps", bufs=2, space="PSUM"))
        acc_scope = stack.enter_context(
            tc.tile_pool(name="acc_ps", bufs=1, space="PSUM"))
        mm_scope = stack.enter_context(
            tc.tile_pool(name="mm_ps", bufs=2, space="PSUM"))
        pt_pool = stack.enter_context(tc.tile_pool(name="pt", bufs=2))
        lr_pool = stack.enter_context(tc.tile_pool(name="lr", bufs=4))

        # ---- constants / weights ----------------------------------------
        wq_sb = consts.tile([128, nkt, inner], CDT)
        wkv_sb = consts.tile([128, nkt, 2 * inner], CDT)
        wout_sb = consts.tile([128, nkt, dim], CDT)
        bout_sb = consts.tile([1, dim], CDT)
        id_sb = consts.tile([128, 128], CDT)
        ones_sb = consts.tile([1, 128], CDT)

        xt_sb = acts.tile([128, nkt, nq], CDT)
        xpt_sb = acts.tile([128, nkt, nk], CDT)
        wq_r = wq_d.ap().rearrange("(t p) o -> p t o", p=128)
        wkv_r = wkv_d.ap().rearrange("(t p) o -> p t o", p=128)
        wout_r = wout_d.ap().rearrange("(t p) o -> p t o", p=128)
        xt_r = xt_d.ap().rearrange("(t p) n -> p t n", p=128)
        xpt_r = xpt_d.ap().rearrange("(t p) n -> p t n", p=128)
        # DMA loads spread over queues, dependency-first. Pool has the
        # cheapest DMA issue cost, so it takes the many small xpt chunks;
        # kt0c0 needs xpt cols 0:512 + wkv K-half, qt0c0 needs xt cols
        # 0:512 + wq -- those go first on their queues.
        # Consolidated multi-dim DMAs (few issues), dependency-first.
        # SP: kt0c0 critical path, then the rest of xpt.
        nc.sync.dma_start(out=xpt_sb[:, :, 0:512], in_=xpt_r[:, :, 0:512])
        nc.sync.dma_start(out=wkv_sb[:, :, 0:inner],
                          in_=wkv_r[:, :, 0:inner])
        nc.sync.dma_start(out=wkv_sb[:, :, inner:2 * inner],
                          in_=wkv_r[:, :, inner:2 * inner])
        nc.sync.dma_start(out=xpt_sb[:, :, 512:1024],
                          in_=xpt_r[:, :, 512:1024])
        nc.sync.dma_start(out=xpt_sb[:, :, 1024:2048],
                          in_=xpt_r[:, :, 1024:2048])
        nc.sync.dma_start(out=id_sb[:, :], in_=id_d.ap())
        nc.sync.dma_start(out=bout_sb[:, :], in_=bout_d.ap())
        # Act: qt0c0 critical path, then free for exp.
        nc.scalar.dma_start(out=wq_sb[:, :, :], in_=wq_r[:, :, :])
        nc.scalar.dma_start(out=xt_sb[:, :, 0:512], in_=xt_r[:, :, 0:512])
        nc.vector.memset(ones_sb[:, :], 1.0)

        qt_sb = acts.tile([128, npr, nq], CDT)    # [inner-slice, nq]
        kt_sb = acts.tile([128, npr, nk], CDT)    # [inner-slice, nk]
        v_sb = acts.tile([128, nj, h * VW], CDT)  # [key-tile, h*(dh+1)]
        ao_sb = acts.tile([128, nq // 128, inner], CDT)   # [q, qtile, inner]
        aotT_sb = acts.tile([128, nkt, nq], CDT)  # [inner-slice, nq]

        for hh in range(h):  # ones columns of V
            nc.vector.memset(v_sb[:, :, hh * VW + DH:hh * VW + DH + 1], 1.0)
        if stub:
            nc.vector.memset(ao_sb[:, :, :], 0.5)
            nc.vector.memset(pt_dummy_guard[:, :], 1.0) if False else None

        # Warm the Exp activation table while DMAs stream in.
        warm_sb = consts.tile([1, 1], FDT)
        nc.scalar.activation(out=warm_sb[:, :], in_=ones_sb[0:1, 0:1],
                             func=Exp, scale=scale)

        # ================= emission scheduler =============================
        PAIRS = [(c, p) for c in range(nqc) for p in range(npr)]
        EST = {"st": 438.0, "pv": 250.0, "mm": 220.0, "tr": 70.0,
               "exp": 1060.0}
        MARGIN = 150.0
        carry = []   # unfinished background chunks handed to the next rep

        for _rep in range(reps):
            state = {"pe": 0.0, "act": 0.0, "v_emitted": 0}
            exp_est = {}

            # ---- background chunks (lazy psum tile, thunk list) ---------
            def kt_thunks(s, cc):
                cell = {}
                for k in range(nkt):
                    def mm(k=k, s=s, cc=cc, cell=cell):
                        if k == 0:
                            cell["ps"] = mm_scope.tile([128, 512], FDT,
                                                       tag="mm", name="mmps")
                        ps = cell["ps"]
                        nc.tensor.matmul(
                            ps[:, :],
                            lhsT=wkv_sb[:, k, s * 128:(s + 1) * 128],
                            rhs=xpt_sb[:, k, cc * 512:(cc + 1) * 512],
                            start=(k == 0), stop=(k == nkt - 1))
                        if k == nkt - 1:
                            nc.vector.tensor_copy(
                                out=kt_sb[:, s, cc * 512:(cc + 1) * 512],
                                in_=ps[:, :])
                    yield (mm, EST["mm"])

            def qt_thunks(s, cc):
                cell = {}
                for k in range(nkt):
                    def mm(k=k, s=s, cc=cc, cell=cell):
                        if k == 0:
                            cell["ps"] = mm_scope.tile([128, 512], FDT,
                                                       tag="mm", name="mmps")
                        ps = cell["ps"]
                        nc.tensor.matmul(
                            ps[:, :],
                            lhsT=wq_sb[:, k, s * 128:(s + 1) * 128],
                            rhs=xt_sb[:, k, cc * 512:(cc + 1) * 512],
                            start=(k == 0), stop=(k == nkt - 1))
                        if k == nkt - 1:
                            nc.vector.tensor_copy(
                                out=qt_sb[:, s, cc * 512:(cc + 1) * 512],
                                in_=ps[:, :])
                    yield (mm, EST["mm"])

            def v_thunks(j):
                cell = {}
                for k in range(nkt):
                    def mm(k=k, j=j, cell=cell):
                        if k == 0:
                            cell["ps"] = mm_scope.tile([128, 512], FDT,
                                                       tag="mm", name="mmps")
                        ps = cell["ps"]
                        nc.tensor.matmul(
                            ps[:, :],
                            lhsT=xpt_sb[:, k, j * 128:(j + 1) * 128],
                            rhs=wkv_sb[:, k, inner:2 * inner],
                            start=(k == 0), stop=(k == nkt - 1))
                        if k == nkt - 1:
                            nc.vector.tensor_copy(
                                out=v_sb[:, j, :].rearrange(
                                    "p (g x) -> p g x", x=VW)[:, :, 0:DH],
                                in_=ps[:, :].rearrange(
                                    "p (g x) -> p g x", x=DH))
                            state["v_emitted"] = j + 1
                    yield (mm, EST["mm"])

            def tr_thunks(c, p):
                if "noout" in stub:
                    return
                cell = {}
                for qq in range(4):
                    def mm(qq=qq, c=c, p=p, cell=cell):
                        if qq == 0:
                            cell["ps"] = mm_scope.tile([128, 512], CDT,
                                                       tag="mm", name="trps")
                        ps = cell["ps"]
                        nc.tensor.matmul(
                            ps[:, qq * 128:(qq + 1) * 128],
                            lhsT=ao_sb[:, c * 4 + qq, p * 128:(p + 1) * 128],
                            rhs=id_sb[:, :], is_transpose=True,
                            start=(qq == 0), stop=(qq == 3))
                        if qq == 3:
                            nc.vector.tensor_copy(
                                out=aotT_sb[:, p, c * 512:(c + 1) * 512],
                                in_=ps[:, :])
                    yield (mm, EST["tr"])

            def out_thunks(c, t):
                if "noout" in stub:
                    def sentinel(t=t):
                        fo = lr_pool.tile([128, dim], FDT, tag="fo", bufs=2,
                                          name="fo")
                        nc.vector.tensor_copy(out=fo[:, :],
                                              in_=st_scope.tile(
                                                  [128, 1024], FDT,
                                                  tag="st",
                                                  name="stx")[:, 0:512])
                        nc.sync.dma_start(
                            out=out_d.ap()[t * 128:(t + 1) * 128, :],
                            in_=fo[:, :])
                    yield (sentinel, EST["mm"])
                    return
                cell = {}
                for k in range(nkt):
                    def mm(k=k, t=t, cell=cell):
                        if k == 0:
                            cell["ps"] = mm_scope.tile([128, 512], FDT,
                                                       tag="mm", name="mmps")
                        ps = cell["ps"]
                        nc.tensor.matmul(
                            ps[:, :],
                            lhsT=aotT_sb[:, k, t * 128:(t + 1) * 128],
                            rhs=wout_sb[:, k, :],
                            start=(k == 0), stop=False)
                    yield (mm, EST["mm"])

                def bias(t=t, cell=cell):
                    ps = cell["ps"]
                    nc.tensor.matmul(ps[:, :], lhsT=ones_sb[:, :],
                                     rhs=bout_sb[:, :], start=False,
                                     stop=True)
                    fo = lr_pool.tile([128, dim], FDT, tag="fo", bufs=2,
                                      name="fo")
                    nc.vector.tensor_copy(out=fo[:, :], in_=ps[:, :])
                    deng = nc.sync if t % 2 == 0 else nc.scalar
                    deng.dma_start(
                        out=out_d.ap()[t * 128:(t + 1) * 128, :], in_=fo[:, :])
                yield (bias, EST["mm"])

            # background FIFO of chunks: [deadline_slot, [thunks], earliest]
            bg = []
            cur = {"chunk": None}  # chunk being emitted (never interleaved)

            def push(deadline, gen, earliest=0):
                bg.append([deadline, list(gen), earliest])

            first = _rep == 0
            for dl, thunks, early in carry:   # previous rep's leftovers
                bg.append([dl, thunks, 0])
            carry.clear()
            # In rep 0 the input DMAs land over the first ~10us, so hold
            # chunks back until their operands exist (earliest slot).
            push(4, kt_thunks(0, 1), 2 if first else 0)
            push(14, v_thunks(0), 1 if first else 0)
            push(14, v_thunks(1), 1 if first else 0)
            push(8, kt_thunks(0, 2), 3 if first else 0)
            push(14, v_thunks(2), 1 if first else 0)
            push(14, v_thunks(3), 1 if first else 0)
            push(12, kt_thunks(0, 3), 3 if first else 0)
            push(18, v_thunks(4), 2 if first else 0)
            push(18, v_thunks(5), 2 if first else 0)
            push(18, v_thunks(6), 2 if first else 0)
            push(16, qt_thunks(1, 0))
            push(16, kt_thunks(1, 0))
            push(22, v_thunks(7), 2 if first else 0)
            push(24, v_thunks(8), 5 if first else 0)
            push(20, kt_thunks(1, 1))
            push(26, v_thunks(9), 5 if first else 0)
            push(28, v_thunks(10), 5 if first else 0)
            push(24, kt_thunks(1, 2))
            push(29, v_thunks(11), 5 if first else 0)
            push(30, v_thunks(12), 6 if first else 0)
            push(28, kt_thunks(1, 3))
            push(30, v_thunks(13), 6 if first else 0)
            push(31, v_thunks(14), 6 if first else 0)
            push(31, v_thunks(15), 6 if first else 0)
            push(32, qt_thunks(2, 0))
            push(32, kt_thunks(2, 0))
            push(36, kt_thunks(2, 1))
            push(40, kt_thunks(2, 2))
            push(44, kt_thunks(2, 3))
            push(48, qt_thunks(3, 0))
            push(48, kt_thunks(3, 0))
            push(52, kt_thunks(3, 1))
            push(56, kt_thunks(3, 2))
            push(60, kt_thunks(3, 3))
            push(64, qt_thunks(0, 1))
            push(80, qt_thunks(1, 1))
            push(96, qt_thunks(2, 1))
            push(108, qt_thunks(3, 1))

            def pop_bg_thunk(slot):
                """Emit one background thunk; returns False if none exists.
                Chunks are atomic: once started, later pops continue it."""
                if cur["chunk"] is None:
                    elig = [i for i, e in enumerate(bg) if e[2] <= slot]
                    if not elig:
                        return False
                    due_i = next((i for i in elig if bg[i][0] <= slot + 1),
                                 None)
                    cur["chunk"] = bg.pop(due_i if due_i is not None
                                          else elig[0])
                fn, est = cur["chunk"][1].pop(0)
                fn()
                state["pe"] += est
                if not cur["chunk"][1]:
                    cur["chunk"] = None
                return True

            def bg_due(slot):
                if cur["chunk"] is not None and cur["chunk"][0] <= slot + 1:
                    return True
                return any(e[0] <= slot + 1 and e[2] <= slot for e in bg)

            # ---- PV stream ----------------------------------------------
            pair_pt = {}
            pair_acc = {}
            pv_queue = [(k, j) for k in range(len(PAIRS)) for j in range(nj)]
            pv_idx = [0]

            def emit_pv(k, j):
                if "nopv" in stub:
                    state["pe"] += EST["pv"]
                    if j == 0:
                        pair_acc[k] = None
                    return
                c, p = PAIRS[k]
                if j == 0:
                    pair_acc[k] = (
                        acc_scope.tile([128, 4, 128], FDT, tag="acc0",
                                       name="acc0"),
                        acc_scope.tile([128, 4, 128], FDT, tag="acc1",
                                       name="acc1"))
                pt = pair_pt[k]
                acc0, acc1 = pair_acc[k]
                # One psum accumulation group per acc bank: start marks the
                # whole 2KB zero region; later qq sub-ranges first-write via
                # the pending-zero overwrite, so only (j0,qq0) starts and
                # (j15,qq3) stops.
                for hh, acc in ((0, acc0), (1, acc1)):
                    h_abs = 2 * p + hh
                    for qq in range(4):
                        nc.tensor.matmul(
                            acc[:, qq, 0:VW],
                            lhsT=pt[:, j, hh * 512 + qq * 128:
                                    hh * 512 + (qq + 1) * 128],
                            rhs=v_sb[:, j, h_abs * VW:(h_abs + 1) * VW],
                            start=(j == 0 and qq == 0),
                            stop=(j == nj - 1 and qq == 3))
                state["pe"] += EST["pv"]

            def emit_normalize(k, slot):
                c, p = PAIRS[k]
                if "nopv" in stub:
                    del pair_acc[k]
                    del pair_pt[k]
                    push(slot + 8, tr_thunks(c, p))
                    if p == npr - 1:
                        for t in range(4):
                            push(slot + 10 + 3 * t, out_thunks(c, 4 * c + t))
                    return
                acc0, acc1 = pair_acc[k]
                for hh, acc in ((0, acc0), (1, acc1)):
                    eng = nc.vector
                    for qq in range(4):
                        r = lr_pool.tile([128, 1], FDT, tag="r", name="rrec")
                        nc.vector.reciprocal(out=r[:, :],
                                             in_=acc[:, qq, DH:DH + 1])
                        eng.tensor_scalar_mul(
                            ao_sb[:, c * 4 + qq,
                                  (2 * p + hh) * DH:(2 * p + hh + 1) * DH],
                            acc[:, qq, 0:DH], r[:, :])
                del pair_acc[k]
                del pair_pt[k]
                push(slot + 8, tr_thunks(c, p))
                if p == npr - 1:
                    for t in range(4):
                        push(slot + 10 + 3 * t, out_thunks(c, 4 * c + t))

            def drain_pv(slot, force_pair_upto=None):
                while pv_idx[0] < len(pv_queue):
                    k, j = pv_queue[pv_idx[0]]
                    if (k, j) not in exp_est:
                        break
                    if state["v_emitted"] <= j:
                        break
                    forced = (force_pair_upto is not None
                              and k <= force_pair_upto)
                    if not forced and (
                            exp_est[(k, j)] > state["pe"] - 50.0
                            or state["pe"] >= state["act"] - MARGIN):
                        break
                    emit_pv(k, j)
                    pv_idx[0] += 1
                    if j == nj - 1:
                        emit_normalize(k, slot)

            # ---- prologue: kt0 chunk0 + qt0 chunk0 ----------------------
            for fn, est in list(kt_thunks(0, 0)) + list(qt_thunks(0, 0)):
                fn()
                state["pe"] += est

            # ---- main slot loop -----------------------------------------
            for k, (c, p) in enumerate(PAIRS):
                if k == 1 and first:
                    # deferred non-critical loads; SP queue is free by now
                    nc.sync.dma_start(out=xt_sb[:, :, 512:1024],
                                      in_=xt_r[:, :, 512:1024])
                    nc.sync.dma_start(out=wout_sb[:, :, :],
                                      in_=wout_r[:, :, :])
                if k >= 2:  # pt buffer rotation (bufs=2)
                    drain_pv(16 * k, force_pair_upto=k - 2)
                    head = (pv_queue[pv_idx[0]][0]
                            if pv_idx[0] < len(pv_queue) else len(PAIRS))
                    assert head > k - 2, "PV stream fell behind pt rotation"
                pair_pt[k] = pt_pool.tile([128, nj, 1024], CDT, tag="pt",
                                          name="ptbuf")
                pt = pair_pt[k]
                for j in range(nj):
                    slot = 16 * k + j
                    st = st_scope.tile([128, 1024], FDT, tag="st")
                    nc.tensor.matmul(
                        st[:, 0:512],
                        lhsT=kt_sb[0:64, p, j * 128:(j + 1) * 128],
                        rhs=qt_sb[0:64, p, c * 512:(c + 1) * 512],
                        start=True, stop=True)
                    nc.tensor.matmul(
                        st[:, 512:1024],
                        lhsT=kt_sb[64:128, p, j * 128:(j + 1) * 128],
                        rhs=qt_sb[64:128, p, c * 512:(c + 1) * 512],
                        start=True, stop=True)
                    state["pe"] += EST["st"]
                    if "noexp" not in stub:
                        nc.scalar.activation(out=pt[:, j, :], in_=st[:, :],
                                             func=Exp, scale=scale)
                    else:
                        nc.vector.tensor_copy(out=pt[:, j, 0:64],
                                              in_=st[:, 0:64])
                    state["act"] = max(state["act"],
                                       state["pe"] + 100.0) + EST["exp"]
                    exp_est[(k, j)] = state["act"]
                    # fill PE slack: overdue bg first, then PV, then bg
                    while bg_due(slot):
                        if not pop_bg_thunk(slot):
                            break
                    drain_pv(slot)
                    while (state["pe"] < state["act"] - MARGIN
                           and pop_bg_thunk(slot)):
                        drain_pv(slot)

            # ---- tail flush ---------------------------------------------
            final_slot = 16 * len(PAIRS)
            drain_pv(final_slot, force_pair_upto=len(PAIRS) - 1)
            assert pv_idx[0] == len(pv_queue), "unemitted PV groups"
            if _rep < reps - 1:
                # hand the remaining chunks (last tr + out c1) to the next
                # rep so they overlap its first exps instead of a dead tail
                if cur["chunk"] is not None:
                    while cur["chunk"] is not None:  # finish current chunk
                        fn, est = cur["chunk"][1].pop(0)
                        fn()
                        if not cur["chunk"][1]:
                            cur["chunk"] = None
                for i, e in enumerate(bg):
                    carry.append([2 + 2 * i, e[1], 0])
                bg.clear()
            else:
                while pop_bg_thunk(10 ** 9):
                    pass
                drain_pv(final_slot, force_pair_upto=len(PAIRS) - 1)
                while pop_bg_thunk(10 ** 9):
                    pass
                assert not bg and cur["chunk"] is None, "unemitted background"

    if compile_module:
        nc.compile()
    return nc


def host_inputs(x, x_prev, w_q, w_kv, w_out, b_out, ncores=NCORES):
    """Shard + lay out the full inputs into per-core input maps."""
    bf16 = ml_dtypes.bfloat16
    b, n, dim = x.shape
    nq = (b * n) // ncores
    halves = ncores // b
    wq = np.ascontiguousarray(w_q).astype(bf16)
    wkv = np.ascontiguousarray(w_kv).astype(bf16)
    wout = np.ascontiguousarray(w_out).astype(bf16)
    bout = np.ascontiguousarray(b_out).reshape(1, dim).astype(bf16)
    ident = np.eye(128, dtype=bf16)
    in_maps = []
    for c in range(ncores):
        bb, half = c // halves, c % halves
        xt = np.ascontiguousarray(
            x[bb, half * nq:(half + 1) * nq, :].T).astype(bf16)
        xpt = np.ascontiguousarray(x_prev[bb].T).astype(bf16)
        in_maps.append(dict(xt=xt, xpt=xpt, wq=wq, wkv=wkv, wout=wout,
                            bout=bout, ident=ident))
    return in_maps


def _get_module():
    global _BUILT
    if _BUILT is None:
        _BUILT = build_module()
    return _BUILT


def kernel(x, x_prev, w_q, w_kv, w_out, b_out):
    from concourse.bass_utils import run_bass_kernel_spmd

    nc = _get_module()
    in_maps = host_inputs(x, x_prev, w_q, w_kv, w_out, b_out)
    res = run_bass_kernel_spmd(nc, in_maps, core_ids=list(range(NCORES)))

    nq = N // 2
    out = np.empty((B, N, DIM), np.float32)
    for c in range(NCORES):
        b, half = c // 2, c % 2
        out[b, half * nq:(half + 1) * nq, :] = res.results[c]["out"]
    return out


# revision 20
# speedup vs baseline: 2.4934x; 1.0795x over previous
"""Multi-head attention (b=4, n=2048, dim=512, h=8, dh=64) on 8 trn2 cores.

Sharding: core c handles batch b=c//2 and query rows
[half*1024, (half+1)*1024) with half=c%2. K/V (from x_prev) are computed
redundantly on both of a batch's cores (cheap vs. attention). No collectives.

Per-core kernel (bf16 operands, fp32 PSUM accumulation):
  QT[inner, nq]  = w_q-tiles  (lhsT) @ x^T          (q in transposed layout)
  KT[inner, nk]  = w_kv-tiles (lhsT) @ x_prev^T
  V [nk, inner]  = x_prev^T-tiles (lhsT) @ w_kv[:, v]  (natural layout,
                                                        + ones column/head)
  ST[j, i]       = KT_h-tile (lhsT, K=dh=64) @ QT_h  (scores transposed;
                   two heads row-tiled in the PE at partitions 0/64)
  PT             = exp(ST * scale)    (no max subtraction: |s*scale| < ~8)
  PVT[q, dh+1]   = sum_j PT_h-tile (lhsT, K=128 keys) @ V_h|ones
                   (transposed-PV form: 65-wide moving operand instead of
                    512-wide -> half the PE row count; col dh = l = sum_j P)
  r[q]           = 1/l  -> AO[q, h dh] = PVT[:, 0:dh] * r   (per-partition
                   scalar broadcast on DVE; no partition-broadcast needed)
  AOT            = PE-transpose(AO) per 128x128 tile  -> [inner, nq] layout
  out[i, d]      = sum_k AOT-tiles (lhsT, K=128) @ w_out-tiles + ones @ b_out

Emission is software-pipelined: per (pair, key-tile) slot the scheduler
emits ST+exp, then drains lagged PVT groups and background work (KT/QT/V
projections, AO transposes, output projection) into the PE slack so the
Activation engine (exp ~= 133us busy, the critical path) is never starved.
Background work is chunked (one psum tile per chunk); chunks may be
reordered by deadline but never interleave with each other, so the tag-
rotated psum buffers see a clean write->drain->write order.
"""

import numpy as np
import ml_dtypes

B, N, DIM = 4, 2048, 512
H, DH, INNER = 8, 64, 512
NCORES = 8

_BUILT = None


def build_module(compile_module=True, reps=1, stub=frozenset(),
                 exp_split=False):
    """Build the per-core attention module (nq=1024, nk=2048 hardcoded).
    reps>1 repeats the whole compute body (timing calibration only)."""
    import concourse.mybir as mybir
    import concourse.tile as tile
    from concourse import bacc

    CDT = mybir.dt.bfloat16
    FDT = mybir.dt.float32
    Exp = mybir.ActivationFunctionType.Exp

    dim, h = DIM, H
    nq, nk = N // 2, N
    inner = h * DH
    nkt = dim // 128          # contraction tiles for projections
    npr = h // 2              # head pairs (= inner // 128 slices of KT/QT)
    nj = nk // 128            # key tiles
    nqc = nq // 512           # query chunks
    VW = DH + 1               # 65: per-head v columns + ones column
    scale = DH ** -0.5

    nc = bacc.Bacc("TRN2", target_bir_lowering=False, debug=False,
                   num_devices=NCORES)

    xt_d = nc.declare_dram_parameter("xt", [dim, nq], CDT, isOutput=False)
    xpt_d = nc.declare_dram_parameter("xpt", [dim, nk], CDT, isOutput=False)
    wq_d = nc.declare_dram_parameter("wq", [dim, inner], CDT, isOutput=False)
    wkv_d = nc.declare_dram_parameter("wkv", [dim, 2 * inner], CDT,
                                      isOutput=False)
    wout_d = nc.declare_dram_parameter("wout", [inner, dim], CDT,
                                       isOutput=False)
    bout_d = nc.declare_dram_parameter("bout", [1, dim], CDT, isOutput=False)
    id_d = nc.declare_dram_parameter("ident", [128, 128], CDT, isOutput=False)
    out_d = nc.declare_dram_parameter("out", [nq, dim], FDT, isOutput=True)

    import contextlib
    with tile.TileContext(nc) as tc, contextlib.ExitStack() as stack:
        consts = stack.enter_context(tc.tile_pool(name="consts", bufs=1))
        acts = stack.enter_context(tc.tile_pool(name="acts", bufs=1))
        st_scope = stack.enter_context(
            tc.tile_pool(name="st_ps", bufs=2, space="PSUM"))
        acc_scope = stack.enter_context(
            tc.tile_pool(name="acc_ps", bufs=1, space="PSUM"))
        mm_scope = stack.enter_context(
            tc.tile_pool(name="mm_ps", bufs=2, space="PSUM"))
        pt_pool = stack.enter_context(tc.tile_pool(name="pt", bufs=2))
        lr_pool = stack.enter_context(tc.tile_pool(name="lr", bufs=4))

        # ---- constants / weights ----------------------------------------
        wq_sb = consts.tile([128, nkt, inner], CDT)
        wkv_sb = consts.tile([128, nkt, 2 * inner], CDT)
        wout_sb = consts.tile([128, nkt, dim], CDT)
        bout_sb = consts.tile([1, dim], CDT)
        id_sb = consts.tile([128, 128], CDT)
        ones_sb = consts.tile([1, 128], CDT)

        xt_sb = acts.tile([128, nkt, nq], CDT)
        xpt_sb = acts.tile([128, nkt, nk], CDT)
        wq_r = wq_d.ap().rearrange("(t p) o -> p t o", p=128)
        wkv_r = wkv_d.ap().rearrange("(t p) o -> p t o", p=128)
        wout_r = wout_d.ap().rearrange("(t p) o -> p t o", p=128)
        xt_r = xt_d.ap().rearrange("(t p) n -> p t n", p=128)
        xpt_r = xpt_d.ap().rearrange("(t p) n -> p t n", p=128)
        # DMA loads spread over queues, dependency-first. Pool has the
        # cheapest DMA issue cost, so it takes the many small xpt chunks;
        # kt0c0 needs xpt cols 0:512 + wkv K-half, qt0c0 needs xt cols
        # 0:512 + wq -- those go first on their queues.
        # Consolidated multi-dim DMAs (few issues), dependency-first.
        # SP: kt0c0 critical path, then the rest of xpt.
        nc.sync.dma_start(out=xpt_sb[:, :, 0:512], in_=xpt_r[:, :, 0:512])
        nc.sync.dma_start(out=wkv_sb[:, :, 0:inner],
                          in_=wkv_r[:, :, 0:inner])
        nc.sync.dma_start(out=wkv_sb[:, :, inner:2 * inner],
                          in_=wkv_r[:, :, inner:2 * inner])
        nc.sync.dma_start(out=xpt_sb[:, :, 512:1024],
                          in_=xpt_r[:, :, 512:1024])
        nc.sync.dma_start(out=xpt_sb[:, :, 1024:2048],
                          in_=xpt_r[:, :, 1024:2048])
        nc.sync.dma_start(out=id_sb[:, :], in_=id_d.ap())
        nc.sync.dma_start(out=bout_sb[:, :], in_=bout_d.ap())
        # Act: qt0c0 critical path, then free for exp.
        nc.scalar.dma_start(out=wq_sb[:, :, :], in_=wq_r[:, :, :])
        nc.scalar.dma_start(out=xt_sb[:, :, 0:512], in_=xt_r[:, :, 0:512])
        nc.vector.memset(ones_sb[:, :], 1.0)

        qt_sb = acts.tile([128, npr, nq], CDT)    # [inner-slice, nq]
        kt_sb = acts.tile([128, npr, nk], CDT)    # [inner-slice, nk]
        v_sb = acts.tile([128, nj, h * VW], CDT)  # [key-tile, h*(dh+1)]
        ao_sb = acts.tile([128, nq // 128, inner], CDT)   # [q, qtile, inner]
        aotT_sb = acts.tile([128, nkt, nq], CDT)  # [inner-slice, nq]

        for hh in range(h):  # ones columns of V
            nc.vector.memset(v_sb[:, :, hh * VW + DH:hh * VW + DH + 1], 1.0)
        if stub:
            nc.vector.memset(ao_sb[:, :, :], 0.5)
            nc.vector.memset(pt_dummy_guard[:, :], 1.0) if False else None

        # Warm the Exp activation table while DMAs stream in.
        warm_sb = consts.tile([1, 1], FDT)
        nc.scalar.activation(out=warm_sb[:, :], in_=ones_sb[0:1, 0:1],
                             func=Exp, scale=scale)

        # ================= emission scheduler =============================
        PAIRS = [(c, p) for c in range(nqc) for p in range(npr)]
        EST = {"st": 438.0, "pv": 250.0, "mm": 220.0, "tr": 70.0,
               "exp": 1060.0}
        MARGIN = 150.0
        carry = []   # unfinished background chunks handed to the next rep

        for _rep in range(reps):
            state = {"pe": 0.0, "act": 0.0, "v_emitted": 0}
            exp_est = {}

            # ---- background chunks (lazy psum tile, thunk list) ---------
            def kt_thunks(s, cc):
                cell = {}
                for k in range(nkt):
                    def mm(k=k, s=s, cc=cc, cell=cell):
                        if k == 0:
                            cell["ps"] = mm_scope.tile([128, 512], FDT,
                                                       tag="mm", name="mmps")
                        ps = cell["ps"]
                        nc.tensor.matmul(
                            ps[:, :],
                            lhsT=wkv_sb[:, k, s * 128:(s + 1) * 128],
                            rhs=xpt_sb[:, k, cc * 512:(cc + 1) * 512],
                            start=(k == 0), stop=(k == nkt - 1))
                        if k == nkt - 1:
                            nc.vector.tensor_copy(
                                out=kt_sb[:, s, cc * 512:(cc + 1) * 512],
                                in_=ps[:, :])
                    yield (mm, EST["mm"])

            def qt_thunks(s, cc):
                cell = {}
                for k in range(nkt):
                    def mm(k=k, s=s, cc=cc, cell=cell):
                        if k == 0:
                            cell["ps"] = mm_scope.tile([128, 512], FDT,
                                                       tag="mm", name="mmps")
                        ps = cell["ps"]
                        nc.tensor.matmul(
                            ps[:, :],
                            lhsT=wq_sb[:, k, s * 128:(s + 1) * 128],
                            rhs=xt_sb[:, k, cc * 512:(cc + 1) * 512],
                            start=(k == 0), stop=(k == nkt - 1))
                        if k == nkt - 1:
                            nc.vector.tensor_copy(
                                out=qt_sb[:, s, cc * 512:(cc + 1) * 512],
                                in_=ps[:, :])
                    yield (mm, EST["mm"])

            def v_thunks(j):
                cell = {}
                for k in range(nkt):
                    def mm(k=k, j=j, cell=cell):
                        if k == 0:
                            cell["ps"] = mm_scope.tile([128, 512], FDT,
                                                       tag="mm", name="mmps")
                        ps = cell["ps"]
                        nc.tensor.matmul(
                            ps[:, :],
                            lhsT=xpt_sb[:, k, j * 128:(j + 1) * 128],
                            rhs=wkv_sb[:, k, inner:2 * inner],
                            start=(k == 0), stop=(k == nkt - 1))
                        if k == nkt - 1:
                            nc.vector.tensor_copy(
                                out=v_sb[:, j, :].rearrange(
                                    "p (g x) -> p g x", x=VW)[:, :, 0:DH],
                                in_=ps[:, :].rearrange(
                                    "p (g x) -> p g x", x=DH))
                            state["v_emitted"] = j + 1
                    yield (mm, EST["mm"])

            def tr_thunks(c, p):
                if "noout" in stub:
                    return
                cell = {}
                for qq in range(4):
                    def mm(qq=qq, c=c, p=p, cell=cell):
                        if qq == 0:
                            cell["ps"] = mm_scope.tile([128, 512], CDT,
                                                       tag="mm", name="trps")
                        ps = cell["ps"]
                        nc.tensor.matmul(
                            ps[:, qq * 128:(qq + 1) * 128],
                            lhsT=ao_sb[:, c * 4 + qq, p * 128:(p + 1) * 128],
                            rhs=id_sb[:, :], is_transpose=True,
                            start=(qq == 0), stop=(qq == 3))
                        if qq == 3:
                            nc.vector.tensor_copy(
                                out=aotT_sb[:, p, c * 512:(c + 1) * 512],
                                in_=ps[:, :])
                    yield (mm, EST["tr"])

            def out_thunks(c, t):
                if "noout" in stub:
                    def sentinel(t=t):
                        fo = lr_pool.tile([128, dim], FDT, tag="fo", bufs=2,
                                          name="fo")
                        nc.vector.tensor_copy(out=fo[:, :],
                                              in_=st_scope.tile(
                                                  [128, 1024], FDT,
                                                  tag="st",
                                                  name="stx")[:, 0:512])
                        nc.sync.dma_start(
                            out=out_d.ap()[t * 128:(t + 1) * 128, :],
                            in_=fo[:, :])
                    yield (sentinel, EST["mm"])
                    return
                cell = {}
                for k in range(nkt):
                    def mm(k=k, t=t, cell=cell):
                        if k == 0:
                            cell["ps"] = mm_scope.tile([128, 512], FDT,
                                                       tag="mm", name="mmps")
                        ps = cell["ps"]
                        nc.tensor.matmul(
                            ps[:, :],
                            lhsT=aotT_sb[:, k, t * 128:(t + 1) * 128],
                            rhs=wout_sb[:, k, :],
                            start=(k == 0), stop=False)
                    yield (mm, EST["mm"])

                def bias(t=t, cell=cell):
                    ps = cell["ps"]
                    nc.tensor.matmul(ps[:, :], lhsT=ones_sb[:, :],
                                     rhs=bout_sb[:, :], start=False,
                                     stop=True)
                    fo = lr_pool.tile([128, dim], FDT, tag="fo", bufs=2,
                                      name="fo")
                    nc.vector.tensor_copy(out=fo[:, :], in_=ps[:, :])
                    deng = nc.sync if t % 2 == 0 else nc.scalar
                    deng.dma_start(
                        out=out_d.ap()[t * 128:(t + 1) * 128, :], in_=fo[:, :])
                yield (bias, EST["mm"])

            # background FIFO of chunks: [deadline_slot, [thunks], earliest]
            bg = []
            cur = {"chunk": None}  # chunk being emitted (never interleaved)

            def push(deadline, gen, earliest=0):
                bg.append([deadline, list(gen), earliest])

            first = _rep == 0
            for dl, thunks, early in carry:   # previous rep's leftovers
                bg.append([dl, thunks, 0])
            carry.clear()
            # In rep 0 the input DMAs land over the first ~10us, so hold
            # chunks back until their operands exist (earliest slot).
            push(4, kt_thunks(0, 1), 2 if first else 0)
            push(14, v_thunks(0), 1 if first else 0)
            push(14, v_thunks(1), 1 if first else 0)
            push(8, kt_thunks(0, 2), 3 if first else 0)
            push(14, v_thunks(2), 1 if first else 0)
            push(14, v_thunks(3), 1 if first else 0)
            push(12, kt_thunks(0, 3), 3 if first else 0)
            push(18, v_thunks(4), 2 if first else 0)
            push(18, v_thunks(5), 2 if first else 0)
            push(18, v_thunks(6), 2 if first else 0)
            push(16, qt_thunks(1, 0))
            push(16, kt_thunks(1, 0))
            push(22, v_thunks(7), 2 if first else 0)
            push(24, v_thunks(8), 5 if first else 0)
            push(20, kt_thunks(1, 1))
            push(26, v_thunks(9), 5 if first else 0)
            push(28, v_thunks(10), 5 if first else 0)
            push(24, kt_thunks(1, 2))
            push(29, v_thunks(11), 5 if first else 0)
            push(30, v_thunks(12), 6 if first else 0)
            push(28, kt_thunks(1, 3))
            push(30, v_thunks(13), 6 if first else 0)
            push(31, v_thunks(14), 6 if first else 0)
            push(31, v_thunks(15), 6 if first else 0)
            push(32, qt_thunks(2, 0))
            push(32, kt_thunks(2, 0))
            push(36, kt_thunks(2, 1))
            push(40, kt_thunks(2, 2))
            push(44, kt_thunks(2, 3))
            push(48, qt_thunks(3, 0))
            push(48, kt_thunks(3, 0))
            push(52, kt_thunks(3, 1))
            push(56, kt_thunks(3, 2))
            push(60, kt_thunks(3, 3))
            push(64, qt_thunks(0, 1))
            push(80, qt_thunks(1, 1))
            push(96, qt_thunks(2, 1))
            push(108, qt_thunks(3, 1))

            def pop_bg_thunk(slot):
                """Emit one background thunk; returns False if none exists.
                Chunks are atomic: once started, later pops continue it."""
                if cur["chunk"] is None:
                    elig = [i for i, e in enumerate(bg) if e[2] <= slot]
                    if not elig:
                        return False
                    due_i = next((i for i in elig if bg[i][0] <= slot + 1),
                                 None)
                    cur["chunk"] = bg.pop(due_i if due_i is not None
                                          else elig[0])
                fn, est = cur["chunk"][1].pop(0)
                fn()
                state["pe"] += est
                if not cur["chunk"][1]:
                    cur["chunk"] = None
                return True

            def bg_due(slot):
                if cur["chunk"] is not None and cur["chunk"][0] <= slot + 1:
                    return True
                return any(e[0] <= slot + 1 and e[2] <= slot for e in bg)

            # ---- PV stream ----------------------------------------------
            pair_pt = {}
            pair_acc = {}
            pv_queue = [(k, j) for k in range(len(PAIRS)) for j in range(nj)]
            pv_idx = [0]

            def emit_pv(k, j):
                if "nopv" in stub:
                    state["pe"] += EST["pv"]
                    if j == 0:
                        pair_acc[k] = None
                    return
                c, p = PAIRS[k]
                if j == 0:
                    pair_acc[k] = (
                        acc_scope.tile([128, 4, 128], FDT, tag="acc0",
                                       name="acc0"),
                        acc_scope.tile([128, 4, 128], FDT, tag="acc1",
                                       name="acc1"))
                pt = pair_pt[k]
                acc0, acc1 = pair_acc[k]
                # One psum accumulation group per acc bank: start marks the
                # whole 2KB zero region; later qq sub-ranges first-write via
                # the pending-zero overwrite, so only (j0,qq0) starts and
                # (j15,qq3) stops.
                for hh, acc in ((0, acc0), (1, acc1)):
                    h_abs = 2 * p + hh
                    for qq in range(4):
                        nc.tensor.matmul(
                            acc[:, qq, 0:VW],
                            lhsT=pt[:, j, hh * 512 + qq * 128:
                                    hh * 512 + (qq + 1) * 128],
                            rhs=v_sb[:, j, h_abs * VW:(h_abs + 1) * VW],
                            start=(j == 0 and qq == 0),
                            stop=(j == nj - 1 and qq == 3))
                state["pe"] += EST["pv"]

            def emit_normalize(k, slot):
                c, p = PAIRS[k]
                if "nopv" in stub:
                    del pair_acc[k]
                    del pair_pt[k]
                    push(slot + 8, tr_thunks(c, p))
                    if p == npr - 1:
                        for t in range(4):
                            push(slot + 10 + 3 * t, out_thunks(c, 4 * c + t))
                    return
                acc0, acc1 = pair_acc[k]
                for hh, acc in ((0, acc0), (1, acc1)):
                    eng = nc.vector
                    for qq in range(4):
                        r = lr_pool.tile([128, 1], FDT, tag="r", name="rrec")
                        nc.vector.reciprocal(out=r[:, :],
                                             in_=acc[:, qq, DH:DH + 1])
                        eng.tensor_scalar_mul(
                            ao_sb[:, c * 4 + qq,
                                  (2 * p + hh) * DH:(2 * p + hh + 1) * DH],
                            acc[:, qq, 0:DH], r[:, :])
                del pair_acc[k]
                del pair_pt[k]
                push(slot + 8, tr_thunks(c, p))
                if p == npr - 1:
                    for t in range(4):
                        push(slot + 10 + 3 * t, out_thunks(c, 4 * c + t))

            def drain_pv(slot, force_pair_upto=None):
                while pv_idx[0] < len(pv_queue):
                    k, j = pv_queue[pv_idx[0]]
                    if (k, j) not in exp_est:
                        break
                    if state["v_emitted"] <= j:
                        break
                    forced = (force_pair_upto is not None
                              and k <= force_pair_upto)
                    if not forced and (
                            exp_est[(k, j)] > state["pe"] - 50.0
                            or state["pe"] >= state["act"] - MARGIN):
                        break
                    emit_pv(k, j)
                    pv_idx[0] += 1
                    if j == nj - 1:
                        emit_normalize(k, slot)

            # ---- prologue: kt0 chunk0 + qt0 chunk0 ----------------------
            for fn, est in list(kt_thunks(0, 0)) + list(qt_thunks(0, 0)):
                fn()
                state["pe"] += est

            # ---- main slot loop -----------------------------------------
            for k, (c, p) in enumerate(PAIRS):
                if k == 1 and first:
                    # deferred non-critical loads; SP queue is free by now
                    nc.sync.dma_start(out=xt_sb[:, :, 512:1024],
                                      in_=xt_r[:, :, 512:1024])
                    nc.sync.dma_start(out=wout_sb[:, :, :],
                                      in_=wout_r[:, :, :])
                if k >= 2:  # pt buffer rotation (bufs=2)
                    drain_pv(16 * k, force_pair_upto=k - 2)
                    head = (pv_queue[pv_idx[0]][0]
                            if pv_idx[0] < len(pv_queue) else len(PAIRS))
                    assert head > k - 2, "PV stream fell behind pt rotation"
                pair_pt[k] = pt_pool.tile([128, nj, 1024], CDT, tag="pt",
                                          name="ptbuf")
                pt = pair_pt[k]
                for j in range(nj):
                    slot = 16 * k + j
                    if exp_split:
                        # two 1-bank st tiles + two 512-wide exps: finer
                        # PE/Act pipelining, slightly cheaper per element
                        sta = st_scope.tile([128, 512], FDT, tag="st",
                                            bufs=4, name="sta")
                        stb = st_scope.tile([128, 512], FDT, tag="st",
                                            bufs=4, name="stb")
                        nc.tensor.matmul(
                            sta[:, :],
                            lhsT=kt_sb[0:64, p, j * 128:(j + 1) * 128],
                            rhs=qt_sb[0:64, p, c * 512:(c + 1) * 512],
                            start=True, stop=True)
                        nc.scalar.activation(out=pt[:, j, 0:512],
                                             in_=sta[:, :],
                                             func=Exp, scale=scale)
                        nc.tensor.matmul(
                            stb[:, :],
                            lhsT=kt_sb[64:128, p, j * 128:(j + 1) * 128],
                            rhs=qt_sb[64:128, p, c * 512:(c + 1) * 512],
                            start=True, stop=True)
                        state["pe"] += EST["st"]
                        nc.scalar.activation(out=pt[:, j, 512:1024],
                                             in_=stb[:, :],
                                             func=Exp, scale=scale)
                    else:
                        st = st_scope.tile([128, 1024], FDT, tag="st")
                        nc.tensor.matmul(
                            st[:, 0:512],
                            lhsT=kt_sb[0:64, p, j * 128:(j + 1) * 128],
                            rhs=qt_sb[0:64, p, c * 512:(c + 1) * 512],
                            start=True, stop=True)
                        nc.tensor.matmul(
                            st[:, 512:1024],
                            lhsT=kt_sb[64:128, p, j * 128:(j + 1) * 128],
                            rhs=qt_sb[64:128, p, c * 512:(c + 1) * 512],
                            start=True, stop=True)
                        state["pe"] += EST["st"]
                        if "noexp" not in stub:
                            nc.scalar.activation(out=pt[:, j, :],
                                                 in_=st[:, :],
                                                 func=Exp, scale=scale)
                        else:
                            nc.vector.tensor_copy(out=pt[:, j, 0:64],
                                                  in_=st[:, 0:64])
                    state["act"] = max(state["act"],
                                       state["pe"] + 100.0) + EST["exp"]
                    exp_est[(k, j)] = state["act"]
                    # fill PE slack: overdue bg first, then PV, then bg
                    while bg_due(slot):
                        if not pop_bg_thunk(slot):
                            break
                    drain_pv(slot)
                    while (state["pe"] < state["act"] - MARGIN
                           and pop_bg_thunk(slot)):
                        drain_pv(slot)

            # ---- tail flush ---------------------------------------------
            final_slot = 16 * len(PAIRS)
            drain_pv(final_slot, force_pair_upto=len(PAIRS) - 1)
            assert pv_idx[0] == len(pv_queue), "unemitted PV groups"
            if _rep < reps - 1:
                # hand the remaining chunks (last tr + out c1) to the next
                # rep so they overlap its first exps instead of a dead tail
                if cur["chunk"] is not None:
                    while cur["chunk"] is not None:  # finish current chunk
                        fn, est = cur["chunk"][1].pop(0)
                        fn()
                        if not cur["chunk"][1]:
                            cur["chunk"] = None
                for i, e in enumerate(bg):
                    carry.append([2 + 2 * i, e[1], 0])
                bg.clear()
            else:
                while pop_bg_thunk(10 ** 9):
                    pass
                drain_pv(final_slot, force_pair_upto=len(PAIRS) - 1)
                while pop_bg_thunk(10 ** 9):
                    pass
                assert not bg and cur["chunk"] is None, "unemitted background"

    if compile_module:
        nc.compile()
    return nc


def host_inputs(x, x_prev, w_q, w_kv, w_out, b_out, ncores=NCORES):
    """Shard + lay out the full inputs into per-core input maps."""
    bf16 = ml_dtypes.bfloat16
    b, n, dim = x.shape
    nq = (b * n) // ncores
    halves = ncores // b
    wq = np.ascontiguousarray(w_q).astype(bf16)
    wkv = np.ascontiguousarray(w_kv).astype(bf16)
    wout = np.ascontiguousarray(w_out).astype(bf16)
    bout = np.ascontiguousarray(b_out).reshape(1, dim).astype(bf16)
    ident = np.eye(128, dtype=bf16)
    in_maps = []
    for c in range(ncores):
        bb, half = c // halves, c % halves
        xt = np.ascontiguousarray(
            x[bb, half * nq:(half + 1) * nq, :].T).astype(bf16)
        xpt = np.ascontiguousarray(x_prev[bb].T).astype(bf16)
        in_maps.append(dict(xt=xt, xpt=xpt, wq=wq, wkv=wkv, wout=wout,
                            bout=bout, ident=ident))
    return in_maps


def _get_module():
    global _BUILT
    if _BUILT is None:
        _BUILT = build_module()
    return _BUILT


def kernel(x, x_prev, w_q, w_kv, w_out, b_out):
    from concourse.bass_utils import run_bass_kernel_spmd

    nc = _get_module()
    in_maps = host_inputs(x, x_prev, w_q, w_kv, w_out, b_out)
    res = run_bass_kernel_spmd(nc, in_maps, core_ids=list(range(NCORES)))

    nq = N // 2
    out = np.empty((B, N, DIM), np.float32)
    for c in range(NCORES):
        b, half = c // 2, c % 2
        out[b, half * nq:(half + 1) * nq, :] = res.results[c]["out"]
    return out
